# revision 11
# baseline (speedup 1.0000x reference)
"""Trainium2 Bass kernel for nn_Bdfdv_51170240364850 (gnn_message_passing).

Computes, for mode pairs (il, im) with im <= il (L1 = 5 modes each way) and
grid (nx=1024, nv=512):

  D[il,im] = base + (-1j)*im*bx*F[il,im] + cB*bm*F[il,im+1]
             + [im==0] Re(cC*bp*F[il,1])
  base     = 0.5*bm*F[il,im-1]  (il>=1, 1<=im<=il)   else  D0[il,im]

with bx = b[:,0], bm = b[:,1]+1j b[:,2], bp = conj(bm),
cB = -(il-im)(il+im+1)/2, cC = -il(il+1).

Strategy: pure data-parallel over nx across 8 NeuronCores (nx=128 per core on
the 128 SBUF partitions), bf16 end-to-end (tolerance 2e-2; bf16 keeps
~7e-3). Per-x scalar products run on the Tensor engine as diagonal-weight
matmuls accumulating in PSUM:

    out[x, :] += c(x) * in[x, :]   ==   PSUM += diag(c).T @ in

Each of the 28 valid output slots (dr/di per (il, im), row 0 on host) is one
PSUM-bank chain of 2-5 matmuls (N=512). Work split that keeps every engine
busy:

 - PE: diagonal matmuls (74 + warm-up), diagonals built on-device from an
   identity tile x f32 coefficient columns (DVE/ACT tensor_scalar, ~200ns).
 - DVE: diag builds, G-feed tensor_scalar ops, and scalar_tensor_tensor
   drains for rows 3-4 that FOLD one product (b0/q2 term) into the
   PSUM->SBUF drain, cutting PE work.
 - ACT: remaining diag builds + plain copy drains (rows 1-2, row 3 im=0 and
   edge).
 - Pool (GpSimd): builds G = F[im-1] + 2cB*F[im+1] (constant integer
   coefficients) for rows 3-4 inner slots via tensor_tensor adds, so each
   inner slot needs only 2 matmuls (h1*G, -/+h2*G~) instead of 4.

Matmuls are emitted in waves of 7 PSUM banks, grouped by diagonal within a
wave to maximize stationary-weight reuse. Output slots are pair-interleaved
and DMA out per (dr,di) pair as soon as drained.
"""

import numpy as np
import ml_dtypes

import bass_rust
import concourse.bass as bass
import concourse.tile as tile
from concourse import mybir
from concourse.bass_utils import run_bass_kernel_spmd

L1 = 5
NX = 1024
NV = 512
NCORES = 8
XS = NX // NCORES  # 128 = SBUF partitions

F32 = mybir.dt.float32
BF16 = mybir.dt.bfloat16
NPBF16 = ml_dtypes.bfloat16

MULT = mybir.AluOpType.mult
ADD = mybir.AluOpType.add

# ---------------------------------------------------------------------------
# coefficient registry
PAIRS = [(2, 1), (3, 1), (3, 2), (4, 1), (4, 2), (4, 3)]  # inner (il, im)


def _cB(il, im):
    return -(il - im) * (il + im + 1) / 2.0


COL = {"one": 0, "h1": 1, "h2p": 2, "h2n": 3}
for _il in range(1, L1):
    COL[("q1", _il)] = 3 + _il           # 4..7
    COL[("r1", _il)] = 7 + _il           # 8..11
    COL[("q2", _il)] = 11 + _il          # 12..15
for _m in range(1, L1):
    COL[("b0p", _m)] = 15 + _m           # 16..19
    COL[("b0n", _m)] = 19 + _m           # 20..23
for _k, _p in enumerate(PAIRS):
    COL[("c1",) + _p] = 24 + 3 * _k
    COL[("c2p",) + _p] = 25 + 3 * _k
    COL[("c2n",) + _p] = 26 + 3 * _k
NSCAL = 48  # padded f32 table width


def coeff_vec(key, b0, b1, b2):
    if key == "one":
        return np.ones_like(b0)
    if key == "h1":
        return 0.5 * b1
    if key == "h2p":
        return 0.5 * b2
    if key == "h2n":
        return -0.5 * b2
    tag = key[0]
    if tag == "q1":
        return 3.0 * _cB(key[1], 0) * b1
    if tag == "r1":
        return _cB(key[1], 0) * b1
    if tag == "q2":
        return _cB(key[1], 0) * b2
    if tag == "b0p":
        return key[1] * b0
    if tag == "b0n":
        return -key[1] * b0
    cB = _cB(key[1], key[2])
    if tag == "c1":
        return cB * b1
    if tag == "c2p":
        return cB * b2
    if tag == "c2n":
        return -cB * b2
    raise ValueError(key)


def build_scal(b_sh):
    """b_sh: [XS, 3] f32 -> [XS, NSCAL] f32 (STT-drain scalars by COL)."""
    b0, b1, b2 = b_sh[:, 0], b_sh[:, 1], b_sh[:, 2]
    s = np.zeros((XS, NSCAL), np.float32)
    for key, c in COL.items():
        s[:, c] = coeff_vec(key, b0, b1, b2)
    return s


# ---------------------------------------------------------------------------
# slot schedule: which products run on PE, which fold into the DVE drain,
# and which inner slots read the Pool-built G tiles.
SLOTS = []
for _il in range(1, L1):
    SLOTS.append((_il, "dr0", 0))
    SLOTS.append((_il, "di0", 0))
    for _im in range(1, _il + 1):
        SLOTS.append((_il, "dr", _im))
        SLOTS.append((_il, "di", _im))

G_ROWS = (2, 3, 4)       # rows whose inner slots use G tiles
G_PAIRS = [(il, im) for il in G_ROWS for im in range(1, il)]
# G adds on Pool except these (DVE tensor_tensor is ~3x faster; used where
# Pool would finish too late or DVE has early slack)
G_TT_DVE = {(2, 1), (3, 1), (3, 2), (4, 1), (4, 2), (4, 3)}


def _is_kill(il, kind, im):
    """STT-drain (DVE) slots: fold the b0/q2 product into the drain."""
    if il == 3 and kind in ("dr", "di") and im < il:
        return True
    return il == 4 and (kind in ("dr0", "di0") or im <= 2)


def _pe_plan(il, kind, im):
    """-> (pe_products, fold or None); products are (col_key, rhs_ref),
    rhs_ref = (space, k) with space in fr/fi/gr/gi/d0r/d0i."""
    kill = _is_kill(il, kind, im)
    if kind == "dr0":
        pe = [("one", ("d0r", 0)), (("q1", il), ("fr", 1))]
        last = (("q2", il), ("fi", 1))
    elif kind == "di0":
        pe = [("one", ("d0i", 0)), (("r1", il), ("fi", 1))]
        last = (("q2", il), ("fr", 1))
    elif kind == "dr":
        if im < il and il in G_ROWS:
            pe = [("h1", ("gr", im)), ("h2n", ("gi", im))]
        elif im < il:
            pe = [("h1", ("fr", im - 1)), ("h2n", ("fi", im - 1)),
                  (("c1", il, im), ("fr", im + 1)),
                  (("c2n", il, im), ("fi", im + 1))]
        else:
            pe = [("h1", ("fr", im - 1)), ("h2n", ("fi", im - 1))]
        last = ((("b0p", im)), ("fi", im))
    elif kind == "di":
        if im < il and il in G_ROWS:
            pe = [("h1", ("gi", im)), ("h2p", ("gr", im))]
        elif im < il:
            pe = [("h1", ("fi", im - 1)), ("h2p", ("fr", im - 1)),
                  (("c1", il, im), ("fi", im + 1)),
                  (("c2p", il, im), ("fr", im + 1))]
        else:
            pe = [("h1", ("fi", im - 1)), ("h2p", ("fr", im - 1))]
        last = ((("b0n", im)), ("fr", im))
    else:
        raise ValueError(kind)
    if kill:
        return pe, last
    return pe + [last], None


# diag keys used on PE, in first-use order
DIAG_ORDER = []
for _s in SLOTS:
    for _ck, _ in _pe_plan(*_s)[0]:
        if _ck not in DIAG_ORDER:
            DIAG_ORDER.append(_ck)
NDIAG = len(DIAG_ORDER)
DIAG_POS = {k: i for i, k in enumerate(DIAG_ORDER)}
NDIAG_DVE = 14           # first N built on DVE, rest on ACT

NDIAG_HOST = min(12, NDIAG)   # first-use diags shipped prebuilt from host

# packed input layout (bf16):
# [identity (128) | host diags (NDIAG_HOST*128) | row blocks il=1..4]
# row block: fr slots (ns), fi slots (ns), d0r, d0i  -- each slot NV cols
HDIAG_OFF = 128
IN_OFF = {}
_o = 128 + NDIAG_HOST * 128
for _il in range(1, L1):
    IN_OFF[_il] = _o
    _o += (2 * (_il + 1) + 2) * NV
CIN = _o

# packed output layout (bf16): row blocks il=1..4, pair-interleaved:
# [dr0, di0, dr1, di1, ...]
OUT_OFF = {}
_o = 0
for _il in range(1, L1):
    OUT_OFF[_il] = _o
    _o += 2 * (_il + 1) * NV
COUT = _o


# ---------------------------------------------------------------------------
# walrus in this container rejects >1 sync-wait per instruction; hoist
# extras onto same-engine NOPs.
def split_multiwaits(nc):
    for f in nc.m.functions:
        for blk in f.blocks:
            new = []
            changed = False
            for ins in blk.instructions:
                si = ins.sync_info
                if si is not None and len(si.on_wait) > 1:
                    waits = list(si.on_wait)
                    for w in waits[:-1]:
                        nop = mybir.InstNoOp(
                            name=nc.get_next_instruction_name(),
                            engine=ins.engine,
                            bass_nofuse=True,
                            sync_info=mybir.SyncInfo(on_wait=[w],
                                                     on_update=[]),
                        )
                        new.append(nop)
                    ins.sync_info = bass_rust.SyncInfo(
                        on_wait=[waits[-1]], on_update=list(si.on_update))
                    changed = True
                new.append(ins)
            if changed:
                blk.instructions = new


def _pair(ap, step_elems, nblocks=2):
    """Contiguous [P, L] AP -> [P, nblocks, L] with element step between
    blocks."""
    c = ap.copy()
    v = c.ap
    last = v.pop()
    v.append((step_elems, nblocks))
    v.append(tuple(last))
    c.ap = v
    return c


# ---------------------------------------------------------------------------
def build_bass(split=True):
    nc = bass.Bass()
    pin = nc.dram_tensor("pin", [XS, CIN], BF16, kind="ExternalInput").ap()
    psc = nc.dram_tensor("psc", [XS, NSCAL], F32, kind="ExternalInput").ap()
    pout = nc.dram_tensor("pout", [XS, COUT], BF16, kind="ExternalOutput").ap()

    with tile.TileContext(nc) as tc:
        with tc.tile_pool(name="m", bufs=1) as pool, \
             tc.tile_pool(name="p", bufs=1, space="PSUM") as ppool:
            scal = pool.tile([XS, NSCAL], F32, tag="scal")
            ident = pool.tile([XS, 128], BF16, tag="ident")
            diags = pool.tile([XS, NDIAG * 128], BF16, tag="diags")

            # issue order doubles as queue priority:
            # scal, ident, host diags, rows 1..4
            nc.sync.dma_start(scal[:], psc[:])
            nc.sync.dma_start(ident[:], pin[:, 0:128])
            nc.sync.dma_start(
                diags[:, 0:NDIAG_HOST * 128],
                pin[:, HDIAG_OFF:HDIAG_OFF + NDIAG_HOST * 128])

            row_in = {}
            for il in range(1, L1):
                ns = il + 1
                t = pool.tile([XS, (2 * ns + 2) * NV], BF16,
                              name=f"in{il}", tag=f"in{il}")
                o = IN_OFF[il]
                if il == 1:
                    # priority part [fr1, fi1, d0r, d0i] lands first so the
                    # im=0 chains can start ~1.5us earlier
                    nc.sync.dma_start(t[:, 0:4 * NV], pin[:, o:o + 4 * NV])
                    nc.sync.dma_start(t[:, 4 * NV:6 * NV],
                                      pin[:, o + 4 * NV:o + 6 * NV])
                else:
                    nc.sync.dma_start(t[:],
                                      pin[:, o:o + (2 * ns + 2) * NV])
                row_in[il] = t

            # remaining diagonals built on-device:
            # diag(col c) = ident * scal[:, c]
            for i, ckey in enumerate(DIAG_ORDER[NDIAG_HOST:],
                                     start=NDIAG_HOST):
                dst = diags[:, i * 128:(i + 1) * 128]
                sc = scal[:, COL[ckey]:COL[ckey] + 1]
                if i < NDIAG_DVE:
                    nc.vector.tensor_scalar_mul(dst, ident[:], sc)
                else:
                    nc.scalar.mul(dst, ident[:], sc)

            # G tiles for rows 3-4 inner slots: G = F[im-1] + 2cB*F[im+1]
            # feed (DVE, imm scalar): P = 2cB * F[im+1]; add (Pool): G = P+F
            g_tile = {}
            p_tile = {}
            for il in G_ROWS:
                ni = il - 1
                g_tile[il] = pool.tile([XS, 2 * ni * NV], BF16,
                                       name=f"g{il}", tag=f"g{il}")
                p_tile[il] = pool.tile([XS, 2 * ni * NV], BF16,
                                       name=f"p{il}", tag=f"p{il}")
            for (il, im) in G_PAIRS:
                ni = il - 1
                ns = il + 1
                t = row_in[il]
                S = ns * NV
                Sg = ni * NV

                def fslot(k):
                    return t[:, k * NV:(k + 1) * NV]

                pslot = p_tile[il][:, (im - 1) * NV:im * NV]
                gslot = g_tile[il][:, (im - 1) * NV:im * NV]
                nc.vector.tensor_scalar_mul(
                    _pair(pslot, Sg), _pair(fslot(im + 1), S),
                    2.0 * _cB(il, im))
                eng = nc.vector if (il, im) in G_TT_DVE else nc.gpsimd
                eng.tensor_tensor(
                    _pair(gslot, Sg), _pair(pslot, Sg),
                    _pair(fslot(im - 1), S), ADD)

            # PE warm-up: ramp the pstate before row-1 data arrives.
            # scratch is never written: the values are irrelevant.
            scratch = pool.tile([XS, NV], BF16, tag="scratch")
            nc.gpsimd.memset(scratch[:], 0)
            wbank = ppool.tile([XS, NV], F32, name="wbank", tag="wbank")
            for _ in range(6):
                nc.tensor.matmul(wbank[:], scratch[:, 0:128], scratch[:],
                                 start=True, stop=True)

            def dg(ckey):
                i = DIAG_POS[ckey]
                return diags[:, i * 128:(i + 1) * 128]

            ROW1_SLOT = {("fr", 1): 0, ("fi", 1): 1, ("d0r", 0): 2,
                         ("d0i", 0): 3, ("fr", 0): 4, ("fi", 0): 5}

            def rhs_ap(il, ref):
                kind, k = ref
                if kind in ("gr", "gi"):
                    ni = il - 1
                    s = (k - 1) if kind == "gr" else (ni + k - 1)
                    return g_tile[il][:, s * NV:(s + 1) * NV]
                t = row_in[il]
                if il == 1:
                    s = ROW1_SLOT[(kind, k)]
                else:
                    ns = il + 1
                    base = {"fr": 0, "fi": ns,
                            "d0r": 2 * ns, "d0i": 2 * ns + 1}
                    s = base[kind] + k
                return t[:, s * NV:(s + 1) * NV]

            row_out = {}
            for il in range(1, L1):
                ns = il + 1
                row_out[il] = pool.tile([XS, 2 * ns * NV], BF16,
                                        name=f"out{il}", tag=f"out{il}")

            def out_ap(il, kind, im):
                # pair-interleaved: slot index = 2*im + (0 if dr else 1)
                s = 2 * im + (0 if kind.startswith("dr") else 1)
                return row_out[il][:, s * NV:(s + 1) * NV]

            banks = [ppool.tile([XS, NV], F32, name=f"bank{i}",
                                tag=f"bank{i}") for i in range(7)]
            banks.append(wbank)

            # emit in waves of 8 slots; group matmuls by diagonal in a wave
            for w0 in range(0, len(SLOTS), 8):
                wave = SLOTS[w0:w0 + 8]
                mms = []          # (diag_pos, bank_idx, ckey, il, ref)
                plans = []
                for j, (il, kind, im) in enumerate(wave):
                    pe, fold = _pe_plan(il, kind, im)
                    plans.append((il, kind, im, fold))
                    for ckey, ref in pe:
                        mms.append((DIAG_POS[ckey], j, ckey, il, ref))
                mms.sort(key=lambda m: m[0])
                remaining = [sum(1 for m in mms if m[1] == j)
                             for j in range(len(wave))]
                started = [False] * len(wave)
                for _, j, ckey, il, ref in mms:
                    remaining[j] -= 1
                    nc.tensor.matmul(banks[j][:], dg(ckey), rhs_ap(il, ref),
                                     start=not started[j],
                                     stop=remaining[j] == 0)
                    started[j] = True
                # drains + pair DMAs in slot order
                for j, (il, kind, im, fold) in enumerate(plans):
                    dst = out_ap(il, kind, im)
                    if fold is not None:
                        ckey, ref = fold
                        c = COL[ckey]
                        nc.vector.scalar_tensor_tensor(
                            dst, rhs_ap(il, ref), scal[:, c:c + 1],
                            banks[j][:], MULT, ADD)
                    else:
                        nc.scalar.copy(dst, banks[j][:])
                    if kind in ("di0", "di"):
                        o = OUT_OFF[il] + 2 * im * NV
                        nc.sync.dma_start(
                            pout[:, o:o + 2 * NV],
                            row_out[il][:, 2 * im * NV:(2 * im + 2) * NV])

    if split:
        split_multiwaits(nc)
    return nc


# ---------------------------------------------------------------------------
def pack_inputs(prev_f_re, prev_f_im, delta0_re, delta0_im, b):
    """-> per-core {'pin': [XS, CIN] bf16, 'psc': [XS, NSCAL] f32}."""
    in_maps = []
    for c in range(NCORES):
        X = slice(c * XS, (c + 1) * XS)
        b_sh = np.asarray(b[X], np.float32)
        p = np.zeros((XS, CIN), NPBF16)
        p[:, 0:128] = np.eye(XS, dtype=NPBF16)
        b0, b1, b2 = b_sh[:, 0], b_sh[:, 1], b_sh[:, 2]
        for i, key in enumerate(DIAG_ORDER[:NDIAG_HOST]):
            d = p[:, HDIAG_OFF + i * 128:HDIAG_OFF + (i + 1) * 128]
            np.fill_diagonal(d, coeff_vec(key, b0, b1, b2).astype(NPBF16))
        def bf(a):
            return np.asarray(a, np.float32).astype(NPBF16)

        # row 1 slot order matches ROW1_SLOT in build_bass
        o = IN_OFF[1]
        for i, a in enumerate([prev_f_re[1, 1, X], prev_f_im[1, 1, X],
                               delta0_re[1, 0, X], delta0_im[1, 0, X],
                               prev_f_re[1, 0, X], prev_f_im[1, 0, X]]):
            p[:, o + i * NV:o + (i + 1) * NV] = bf(a)
        for il in range(2, L1):
            o = IN_OFF[il]
            ns = il + 1
            p[:, o:o + ns * NV] = (
                np.asarray(prev_f_re[il, :ns, X, :], np.float32)
                .transpose(1, 0, 2).reshape(XS, ns * NV).astype(NPBF16))
            o += ns * NV
            p[:, o:o + ns * NV] = (
                np.asarray(prev_f_im[il, :ns, X, :], np.float32)
                .transpose(1, 0, 2).reshape(XS, ns * NV).astype(NPBF16))
            o += ns * NV
            p[:, o:o + NV] = np.asarray(
                delta0_re[il, 0, X, :], np.float32).astype(NPBF16)
            o += NV
            p[:, o:o + NV] = np.asarray(
                delta0_im[il, 0, X, :], np.float32).astype(NPBF16)
        in_maps.append({"pin": p, "psc": build_scal(b_sh)})
    return in_maps


def unpack_outputs(results, delta0_re, delta0_im):
    out = np.zeros((L1, L1, NX, NV), np.complex64)
    out[0, 0] = np.asarray(delta0_re[0, 0]) + 1j * np.asarray(delta0_im[0, 0])
    for c in range(NCORES):
        X = slice(c * XS, (c + 1) * XS)
        p = results[c]["pout"].astype(np.float32)
        for il in range(1, L1):
            o = OUT_OFF[il]
            ns = il + 1
            blk = p[:, o:o + 2 * ns * NV].reshape(XS, ns, 2, NV)
            dr = blk[:, :, 0, :].transpose(1, 0, 2)
            di = blk[:, :, 1, :].transpose(1, 0, 2)
            out[il, :ns, X, :] = dr + 1j * di
    return out


_NC_CACHE = None


def get_nc():
    global _NC_CACHE
    if _NC_CACHE is None:
        _NC_CACHE = build_bass()
    return _NC_CACHE


def kernel(prev_f_re, prev_f_im, delta0_re, delta0_im, b, v):
    in_maps = pack_inputs(prev_f_re, prev_f_im, delta0_re, delta0_im, b)
    res = run_bass_kernel_spmd(get_nc(), in_maps, list(range(NCORES)))
    return unpack_outputs(res.results, delta0_re, delta0_im)


# revision 12
# speedup vs baseline: 1.0037x; 1.0037x over previous
"""Trainium2 Bass kernel for nn_Bdfdv_51170240364850 (gnn_message_passing).

Computes, for mode pairs (il, im) with im <= il (L1 = 5 modes each way) and
grid (nx=1024, nv=512):

  D[il,im] = base + (-1j)*im*bx*F[il,im] + cB*bm*F[il,im+1]
             + [im==0] Re(cC*bp*F[il,1])
  base     = 0.5*bm*F[il,im-1]  (il>=1, 1<=im<=il)   else  D0[il,im]

with bx = b[:,0], bm = b[:,1]+1j b[:,2], bp = conj(bm),
cB = -(il-im)(il+im+1)/2, cC = -il(il+1).

Strategy: pure data-parallel over nx across 8 NeuronCores (nx=128 per core on
the 128 SBUF partitions), bf16 end-to-end (tolerance 2e-2; bf16 keeps
~7e-3). Per-x scalar products run on the Tensor engine as diagonal-weight
matmuls accumulating in PSUM:

    out[x, :] += c(x) * in[x, :]   ==   PSUM += diag(c).T @ in

Each of the 28 valid output slots (dr/di per (il, im), row 0 on host) is one
PSUM-bank chain of 2-5 matmuls (N=512). Work split that keeps every engine
busy:

 - PE: diagonal matmuls (74 + warm-up), diagonals built on-device from an
   identity tile x f32 coefficient columns (DVE/ACT tensor_scalar, ~200ns).
 - DVE: diag builds, G-feed tensor_scalar ops, and scalar_tensor_tensor
   drains for rows 3-4 that FOLD one product (b0/q2 term) into the
   PSUM->SBUF drain, cutting PE work.
 - ACT: remaining diag builds + plain copy drains (rows 1-2, row 3 im=0 and
   edge).
 - Pool (GpSimd): builds G = F[im-1] + 2cB*F[im+1] (constant integer
   coefficients) for rows 3-4 inner slots via tensor_tensor adds, so each
   inner slot needs only 2 matmuls (h1*G, -/+h2*G~) instead of 4.

Matmuls are emitted in waves of 7 PSUM banks, grouped by diagonal within a
wave to maximize stationary-weight reuse. Output slots are pair-interleaved
and DMA out per (dr,di) pair as soon as drained.
"""

import numpy as np
import ml_dtypes

import bass_rust
import concourse.bass as bass
import concourse.tile as tile
from concourse import mybir
from concourse.bass_utils import run_bass_kernel_spmd

L1 = 5
NX = 1024
NV = 512
NCORES = 8
XS = NX // NCORES  # 128 = SBUF partitions

F32 = mybir.dt.float32
BF16 = mybir.dt.bfloat16
NPBF16 = ml_dtypes.bfloat16

MULT = mybir.AluOpType.mult
ADD = mybir.AluOpType.add

# ---------------------------------------------------------------------------
# coefficient registry
PAIRS = [(2, 1), (3, 1), (3, 2), (4, 1), (4, 2), (4, 3)]  # inner (il, im)


def _cB(il, im):
    return -(il - im) * (il + im + 1) / 2.0


COL = {"one": 0, "h1": 1, "h2p": 2, "h2n": 3}
for _il in range(1, L1):
    COL[("q1", _il)] = 3 + _il           # 4..7
    COL[("r1", _il)] = 7 + _il           # 8..11
    COL[("q2", _il)] = 11 + _il          # 12..15
for _m in range(1, L1):
    COL[("b0p", _m)] = 15 + _m           # 16..19
    COL[("b0n", _m)] = 19 + _m           # 20..23
for _k, _p in enumerate(PAIRS):
    COL[("c1",) + _p] = 24 + 3 * _k
    COL[("c2p",) + _p] = 25 + 3 * _k
    COL[("c2n",) + _p] = 26 + 3 * _k
NSCAL = 48  # padded f32 table width


def coeff_vec(key, b0, b1, b2):
    if key == "one":
        return np.ones_like(b0)
    if key == "h1":
        return 0.5 * b1
    if key == "h2p":
        return 0.5 * b2
    if key == "h2n":
        return -0.5 * b2
    tag = key[0]
    if tag == "q1":
        return 3.0 * _cB(key[1], 0) * b1
    if tag == "r1":
        return _cB(key[1], 0) * b1
    if tag == "q2":
        return _cB(key[1], 0) * b2
    if tag == "b0p":
        return key[1] * b0
    if tag == "b0n":
        return -key[1] * b0
    cB = _cB(key[1], key[2])
    if tag == "c1":
        return cB * b1
    if tag == "c2p":
        return cB * b2
    if tag == "c2n":
        return -cB * b2
    raise ValueError(key)


def build_scal(b_sh):
    """b_sh: [XS, 3] f32 -> [XS, NSCAL] f32 (STT-drain scalars by COL)."""
    b0, b1, b2 = b_sh[:, 0], b_sh[:, 1], b_sh[:, 2]
    s = np.zeros((XS, NSCAL), np.float32)
    for key, c in COL.items():
        s[:, c] = coeff_vec(key, b0, b1, b2)
    return s


# ---------------------------------------------------------------------------
# slot schedule: which products run on PE, which fold into the DVE drain,
# and which inner slots read the Pool-built G tiles.
SLOTS = []
for _il in range(1, L1):
    SLOTS.append((_il, "dr0", 0))
    SLOTS.append((_il, "di0", 0))
    for _im in range(1, _il + 1):
        SLOTS.append((_il, "dr", _im))
        SLOTS.append((_il, "di", _im))

G_ROWS = (2, 3, 4)       # rows whose inner slots use G tiles
G_PAIRS = [(il, im) for il in G_ROWS for im in range(1, il)]
# G adds on Pool except these (DVE tensor_tensor is ~3x faster; used where
# Pool would finish too late or DVE has early slack)
G_TT_DVE = {(2, 1), (3, 1), (3, 2), (4, 1), (4, 2), (4, 3)}


def _is_kill(il, kind, im):
    """STT-drain (DVE) slots: fold the b0/q2 product into the drain."""
    if il == 3 and kind in ("dr", "di") and im < il:
        return True
    return il == 4 and (kind in ("dr0", "di0") or im <= 2)


def _pe_plan(il, kind, im):
    """-> (pe_products, fold or None); products are (col_key, rhs_ref),
    rhs_ref = (space, k) with space in fr/fi/gr/gi/d0r/d0i."""
    kill = _is_kill(il, kind, im)
    if kind == "dr0":
        pe = [("one", ("d0r", 0)), (("q1", il), ("fr", 1))]
        last = (("q2", il), ("fi", 1))
    elif kind == "di0":
        pe = [("one", ("d0i", 0)), (("r1", il), ("fi", 1))]
        last = (("q2", il), ("fr", 1))
    elif kind == "dr":
        if im < il and il in G_ROWS:
            pe = [("h1", ("gr", im)), ("h2n", ("gi", im))]
        elif im < il:
            pe = [("h1", ("fr", im - 1)), ("h2n", ("fi", im - 1)),
                  (("c1", il, im), ("fr", im + 1)),
                  (("c2n", il, im), ("fi", im + 1))]
        else:
            pe = [("h1", ("fr", im - 1)), ("h2n", ("fi", im - 1))]
        last = ((("b0p", im)), ("fi", im))
    elif kind == "di":
        if im < il and il in G_ROWS:
            pe = [("h1", ("gi", im)), ("h2p", ("gr", im))]
        elif im < il:
            pe = [("h1", ("fi", im - 1)), ("h2p", ("fr", im - 1)),
                  (("c1", il, im), ("fi", im + 1)),
                  (("c2p", il, im), ("fr", im + 1))]
        else:
            pe = [("h1", ("fi", im - 1)), ("h2p", ("fr", im - 1))]
        last = ((("b0n", im)), ("fr", im))
    else:
        raise ValueError(kind)
    if kill:
        return pe, last
    return pe + [last], None


# diag keys used on PE, in first-use order
DIAG_ORDER = []
for _s in SLOTS:
    for _ck, _ in _pe_plan(*_s)[0]:
        if _ck not in DIAG_ORDER:
            DIAG_ORDER.append(_ck)
NDIAG = len(DIAG_ORDER)
DIAG_POS = {k: i for i, k in enumerate(DIAG_ORDER)}
NDIAG_DVE = 14           # first N built on DVE, rest on ACT

# packed input layout (bf16): [identity (128 cols) | row blocks il=1..4]
# row block: fr slots (ns), fi slots (ns), d0r, d0i  -- each slot NV cols
IN_OFF = {}
_o = 128
for _il in range(1, L1):
    IN_OFF[_il] = _o
    _o += (2 * (_il + 1) + 2) * NV
CIN = _o

# packed output layout (bf16): row blocks il=1..4, pair-interleaved:
# [dr0, di0, dr1, di1, ...]
OUT_OFF = {}
_o = 0
for _il in range(1, L1):
    OUT_OFF[_il] = _o
    _o += 2 * (_il + 1) * NV
COUT = _o


# ---------------------------------------------------------------------------
# walrus in this container rejects >1 sync-wait per instruction; hoist
# extras onto same-engine NOPs.
def split_multiwaits(nc):
    for f in nc.m.functions:
        for blk in f.blocks:
            new = []
            changed = False
            for ins in blk.instructions:
                si = ins.sync_info
                if si is not None and len(si.on_wait) > 1:
                    waits = list(si.on_wait)
                    for w in waits[:-1]:
                        nop = mybir.InstNoOp(
                            name=nc.get_next_instruction_name(),
                            engine=ins.engine,
                            bass_nofuse=True,
                            sync_info=mybir.SyncInfo(on_wait=[w],
                                                     on_update=[]),
                        )
                        new.append(nop)
                    ins.sync_info = bass_rust.SyncInfo(
                        on_wait=[waits[-1]], on_update=list(si.on_update))
                    changed = True
                new.append(ins)
            if changed:
                blk.instructions = new


def _pair(ap, step_elems, nblocks=2):
    """Contiguous [P, L] AP -> [P, nblocks, L] with element step between
    blocks."""
    c = ap.copy()
    v = c.ap
    last = v.pop()
    v.append((step_elems, nblocks))
    v.append(tuple(last))
    c.ap = v
    return c


# ---------------------------------------------------------------------------
def build_bass(split=True):
    nc = bass.Bass()
    pin = nc.dram_tensor("pin", [XS, CIN], BF16, kind="ExternalInput").ap()
    psc = nc.dram_tensor("psc", [XS, NSCAL], F32, kind="ExternalInput").ap()
    pout = nc.dram_tensor("pout", [XS, COUT], BF16, kind="ExternalOutput").ap()

    with tile.TileContext(nc) as tc:
        with tc.tile_pool(name="m", bufs=1) as pool, \
             tc.tile_pool(name="p", bufs=1, space="PSUM") as ppool:
            scal = pool.tile([XS, NSCAL], F32, tag="scal")
            ident = pool.tile([XS, 128], BF16, tag="ident")
            diags = pool.tile([XS, NDIAG * 128], BF16, tag="diags")

            # issue order doubles as queue priority: scal, ident, rows 1..4
            nc.sync.dma_start(scal[:], psc[:])
            nc.sync.dma_start(ident[:], pin[:, 0:128])

            row_in = {}
            for il in range(1, L1):
                ns = il + 1
                t = pool.tile([XS, (2 * ns + 2) * NV], BF16,
                              name=f"in{il}", tag=f"in{il}")
                o = IN_OFF[il]
                if il == 1:
                    # priority part [fr1, fi1, d0r, d0i] lands first so the
                    # im=0 chains can start ~1.5us earlier
                    nc.sync.dma_start(t[:, 0:4 * NV], pin[:, o:o + 4 * NV])
                    nc.sync.dma_start(t[:, 4 * NV:6 * NV],
                                      pin[:, o + 4 * NV:o + 6 * NV])
                else:
                    nc.sync.dma_start(t[:],
                                      pin[:, o:o + (2 * ns + 2) * NV])
                row_in[il] = t

            # on-device diagonal weights: diag(col c) = ident * scal[:, c]
            for i, ckey in enumerate(DIAG_ORDER):
                dst = diags[:, i * 128:(i + 1) * 128]
                sc = scal[:, COL[ckey]:COL[ckey] + 1]
                if i < NDIAG_DVE:
                    nc.vector.tensor_scalar_mul(dst, ident[:], sc)
                else:
                    nc.scalar.mul(dst, ident[:], sc)

            # G tiles for rows 3-4 inner slots: G = F[im-1] + 2cB*F[im+1]
            # feed (DVE, imm scalar): P = 2cB * F[im+1]; add (Pool): G = P+F
            g_tile = {}
            p_tile = {}
            for il in G_ROWS:
                ni = il - 1
                g_tile[il] = pool.tile([XS, 2 * ni * NV], BF16,
                                       name=f"g{il}", tag=f"g{il}")
                p_tile[il] = pool.tile([XS, 2 * ni * NV], BF16,
                                       name=f"p{il}", tag=f"p{il}")
            for (il, im) in G_PAIRS:
                ni = il - 1
                ns = il + 1
                t = row_in[il]
                S = ns * NV
                Sg = ni * NV

                def fslot(k):
                    return t[:, k * NV:(k + 1) * NV]

                pslot = p_tile[il][:, (im - 1) * NV:im * NV]
                gslot = g_tile[il][:, (im - 1) * NV:im * NV]
                nc.vector.tensor_scalar_mul(
                    _pair(pslot, Sg), _pair(fslot(im + 1), S),
                    2.0 * _cB(il, im))
                eng = nc.vector if (il, im) in G_TT_DVE else nc.gpsimd
                eng.tensor_tensor(
                    _pair(gslot, Sg), _pair(pslot, Sg),
                    _pair(fslot(im - 1), S), ADD)

            # PE warm-up: ramp the pstate before row-1 data arrives.
            # scratch is never written: the values are irrelevant.
            scratch = pool.tile([XS, NV], BF16, tag="scratch")
            nc.gpsimd.memset(scratch[:], 0)
            wbank = ppool.tile([XS, NV], F32, name="wbank", tag="wbank")
            for _ in range(6):
                nc.tensor.matmul(wbank[:], scratch[:, 0:128], scratch[:],
                                 start=True, stop=True)

            def dg(ckey):
                i = DIAG_POS[ckey]
                return diags[:, i * 128:(i + 1) * 128]

            ROW1_SLOT = {("fr", 1): 0, ("fi", 1): 1, ("d0r", 0): 2,
                         ("d0i", 0): 3, ("fr", 0): 4, ("fi", 0): 5}

            def rhs_ap(il, ref):
                kind, k = ref
                if kind in ("gr", "gi"):
                    ni = il - 1
                    s = (k - 1) if kind == "gr" else (ni + k - 1)
                    return g_tile[il][:, s * NV:(s + 1) * NV]
                t = row_in[il]
                if il == 1:
                    s = ROW1_SLOT[(kind, k)]
                else:
                    ns = il + 1
                    base = {"fr": 0, "fi": ns,
                            "d0r": 2 * ns, "d0i": 2 * ns + 1}
                    s = base[kind] + k
                return t[:, s * NV:(s + 1) * NV]

            row_out = {}
            for il in range(1, L1):
                ns = il + 1
                row_out[il] = pool.tile([XS, 2 * ns * NV], BF16,
                                        name=f"out{il}", tag=f"out{il}")

            def out_ap(il, kind, im):
                # pair-interleaved: slot index = 2*im + (0 if dr else 1)
                s = 2 * im + (0 if kind.startswith("dr") else 1)
                return row_out[il][:, s * NV:(s + 1) * NV]

            banks = [ppool.tile([XS, NV], F32, name=f"bank{i}",
                                tag=f"bank{i}") for i in range(7)]
            banks.append(wbank)

            # emit in waves of 8 slots; group matmuls by diagonal in a wave
            for w0 in range(0, len(SLOTS), 8):
                wave = SLOTS[w0:w0 + 8]
                mms = []          # (diag_pos, bank_idx, ckey, il, ref)
                plans = []
                for j, (il, kind, im) in enumerate(wave):
                    pe, fold = _pe_plan(il, kind, im)
                    plans.append((il, kind, im, fold))
                    for ckey, ref in pe:
                        mms.append((DIAG_POS[ckey], j, ckey, il, ref))
                mms.sort(key=lambda m: m[0])
                remaining = [sum(1 for m in mms if m[1] == j)
                             for j in range(len(wave))]
                started = [False] * len(wave)
                for _, j, ckey, il, ref in mms:
                    remaining[j] -= 1
                    nc.tensor.matmul(banks[j][:], dg(ckey), rhs_ap(il, ref),
                                     start=not started[j],
                                     stop=remaining[j] == 0)
                    started[j] = True
                # drains + pair DMAs in slot order
                for j, (il, kind, im, fold) in enumerate(plans):
                    dst = out_ap(il, kind, im)
                    if fold is not None:
                        ckey, ref = fold
                        c = COL[ckey]
                        nc.vector.scalar_tensor_tensor(
                            dst, rhs_ap(il, ref), scal[:, c:c + 1],
                            banks[j][:], MULT, ADD)
                    else:
                        nc.scalar.copy(dst, banks[j][:])
                    if kind in ("di0", "di"):
                        o = OUT_OFF[il] + 2 * im * NV
                        nc.sync.dma_start(
                            pout[:, o:o + 2 * NV],
                            row_out[il][:, 2 * im * NV:(2 * im + 2) * NV])

    if split:
        split_multiwaits(nc)
    return nc


# ---------------------------------------------------------------------------
def pack_inputs(prev_f_re, prev_f_im, delta0_re, delta0_im, b):
    """-> per-core {'pin': [XS, CIN] bf16, 'psc': [XS, NSCAL] f32}."""
    in_maps = []
    for c in range(NCORES):
        X = slice(c * XS, (c + 1) * XS)
        b_sh = np.asarray(b[X], np.float32)
        p = np.zeros((XS, CIN), NPBF16)
        p[:, 0:128] = np.eye(XS, dtype=NPBF16)
        def bf(a):
            return np.asarray(a, np.float32).astype(NPBF16)

        # row 1 slot order matches ROW1_SLOT in build_bass
        o = IN_OFF[1]
        for i, a in enumerate([prev_f_re[1, 1, X], prev_f_im[1, 1, X],
                               delta0_re[1, 0, X], delta0_im[1, 0, X],
                               prev_f_re[1, 0, X], prev_f_im[1, 0, X]]):
            p[:, o + i * NV:o + (i + 1) * NV] = bf(a)
        for il in range(2, L1):
            o = IN_OFF[il]
            ns = il + 1
            p[:, o:o + ns * NV] = (
                np.asarray(prev_f_re[il, :ns, X, :], np.float32)
                .transpose(1, 0, 2).reshape(XS, ns * NV).astype(NPBF16))
            o += ns * NV
            p[:, o:o + ns * NV] = (
                np.asarray(prev_f_im[il, :ns, X, :], np.float32)
                .transpose(1, 0, 2).reshape(XS, ns * NV).astype(NPBF16))
            o += ns * NV
            p[:, o:o + NV] = np.asarray(
                delta0_re[il, 0, X, :], np.float32).astype(NPBF16)
            o += NV
            p[:, o:o + NV] = np.asarray(
                delta0_im[il, 0, X, :], np.float32).astype(NPBF16)
        in_maps.append({"pin": p, "psc": build_scal(b_sh)})
    return in_maps


def unpack_outputs(results, delta0_re, delta0_im):
    out = np.zeros((L1, L1, NX, NV), np.complex64)
    out[0, 0] = np.asarray(delta0_re[0, 0]) + 1j * np.asarray(delta0_im[0, 0])
    for c in range(NCORES):
        X = slice(c * XS, (c + 1) * XS)
        p = results[c]["pout"].astype(np.float32)
        for il in range(1, L1):
            o = OUT_OFF[il]
            ns = il + 1
            blk = p[:, o:o + 2 * ns * NV].reshape(XS, ns, 2, NV)
            dr = blk[:, :, 0, :].transpose(1, 0, 2)
            di = blk[:, :, 1, :].transpose(1, 0, 2)
            out[il, :ns, X, :] = dr + 1j * di
    return out


_NC_CACHE = None


def get_nc():
    global _NC_CACHE
    if _NC_CACHE is None:
        _NC_CACHE = build_bass()
    return _NC_CACHE


def kernel(prev_f_re, prev_f_im, delta0_re, delta0_im, b, v):
    in_maps = pack_inputs(prev_f_re, prev_f_im, delta0_re, delta0_im, b)
    res = run_bass_kernel_spmd(get_nc(), in_maps, list(range(NCORES)))
    return unpack_outputs(res.results, delta0_re, delta0_im)


# revision 13
# speedup vs baseline: 1.0480x; 1.0442x over previous
"""Trainium2 Bass kernel for nn_Bdfdv_51170240364850 (gnn_message_passing).

Computes, for mode pairs (il, im) with im <= il (L1 = 5 modes each way) and
grid (nx=1024, nv=512):

  D[il,im] = base + (-1j)*im*bx*F[il,im] + cB*bm*F[il,im+1]
             + [im==0] Re(cC*bp*F[il,1])
  base     = 0.5*bm*F[il,im-1]  (il>=1, 1<=im<=il)   else  D0[il,im]

with bx = b[:,0], bm = b[:,1]+1j b[:,2], bp = conj(bm),
cB = -(il-im)(il+im+1)/2, cC = -il(il+1).

Strategy: pure data-parallel over nx across 8 NeuronCores (nx=128 per core on
the 128 SBUF partitions), bf16 end-to-end (tolerance 2e-2; bf16 keeps
~7e-3). Per-x scalar products run on the Tensor engine as diagonal-weight
matmuls accumulating in PSUM:

    out[x, :] += c(x) * in[x, :]   ==   PSUM += diag(c).T @ in

Each of the 28 valid output slots (dr/di per (il, im), row 0 on host) is one
PSUM-bank chain of 2-5 matmuls (N=512). Work split that keeps every engine
busy:

 - PE: diagonal matmuls (74 + warm-up), diagonals built on-device from an
   identity tile x f32 coefficient columns (DVE/ACT tensor_scalar, ~200ns).
 - DVE: diag builds, G-feed tensor_scalar ops, and scalar_tensor_tensor
   drains for rows 3-4 that FOLD one product (b0/q2 term) into the
   PSUM->SBUF drain, cutting PE work.
 - ACT: remaining diag builds + plain copy drains (rows 1-2, row 3 im=0 and
   edge).
 - Pool (GpSimd): builds G = F[im-1] + 2cB*F[im+1] (constant integer
   coefficients) for rows 3-4 inner slots via tensor_tensor adds, so each
   inner slot needs only 2 matmuls (h1*G, -/+h2*G~) instead of 4.

Matmuls are emitted in waves of 7 PSUM banks, grouped by diagonal within a
wave to maximize stationary-weight reuse. Output slots are pair-interleaved
and DMA out per (dr,di) pair as soon as drained.
"""

import numpy as np
import ml_dtypes

import bass_rust
import concourse.bass as bass
import concourse.tile as tile
from concourse import mybir
from concourse.bass_utils import run_bass_kernel_spmd

L1 = 5
NX = 1024
NV = 512
NCORES = 8
XS = NX // NCORES  # 128 = SBUF partitions

F32 = mybir.dt.float32
BF16 = mybir.dt.bfloat16
NPBF16 = ml_dtypes.bfloat16

MULT = mybir.AluOpType.mult
ADD = mybir.AluOpType.add

# ---------------------------------------------------------------------------
# coefficient registry
PAIRS = [(2, 1), (3, 1), (3, 2), (4, 1), (4, 2), (4, 3)]  # inner (il, im)


def _cB(il, im):
    return -(il - im) * (il + im + 1) / 2.0


COL = {"one": 0, "h1": 1, "h2p": 2, "h2n": 3}
for _il in range(1, L1):
    COL[("q1", _il)] = 3 + _il           # 4..7
    COL[("r1", _il)] = 7 + _il           # 8..11
    COL[("q2", _il)] = 11 + _il          # 12..15
for _m in range(1, L1):
    COL[("b0p", _m)] = 15 + _m           # 16..19
    COL[("b0n", _m)] = 19 + _m           # 20..23
for _k, _p in enumerate(PAIRS):
    COL[("c1",) + _p] = 24 + 3 * _k
    COL[("c2p",) + _p] = 25 + 3 * _k
    COL[("c2n",) + _p] = 26 + 3 * _k
NSCAL = 48  # padded f32 table width


def coeff_vec(key, b0, b1, b2):
    if key == "one":
        return np.ones_like(b0)
    if key == "h1":
        return 0.5 * b1
    if key == "h2p":
        return 0.5 * b2
    if key == "h2n":
        return -0.5 * b2
    tag = key[0]
    if tag == "q1":
        return 3.0 * _cB(key[1], 0) * b1
    if tag == "r1":
        return _cB(key[1], 0) * b1
    if tag == "q2":
        return _cB(key[1], 0) * b2
    if tag == "b0p":
        return key[1] * b0
    if tag == "b0n":
        return -key[1] * b0
    cB = _cB(key[1], key[2])
    if tag == "c1":
        return cB * b1
    if tag == "c2p":
        return cB * b2
    if tag == "c2n":
        return -cB * b2
    raise ValueError(key)


def build_scal(b_sh):
    """b_sh: [XS, 3] f32 -> [XS, NSCAL] f32 (STT-drain scalars by COL)."""
    b0, b1, b2 = b_sh[:, 0], b_sh[:, 1], b_sh[:, 2]
    s = np.zeros((XS, NSCAL), np.float32)
    for key, c in COL.items():
        s[:, c] = coeff_vec(key, b0, b1, b2)
    return s


# ---------------------------------------------------------------------------
# slot schedule: which products run on PE, which fold into the DVE drain,
# and which inner slots read the Pool-built G tiles.
SLOTS = []
for _il in range(1, L1):
    SLOTS.append((_il, "dr0", 0))
    SLOTS.append((_il, "di0", 0))
    for _im in range(1, _il + 1):
        SLOTS.append((_il, "dr", _im))
        SLOTS.append((_il, "di", _im))

G_ROWS = (2, 3, 4)       # rows whose inner slots use G tiles
G_PAIRS = [(il, im) for il in G_ROWS for im in range(1, il)]
# G adds on Pool except these (DVE tensor_tensor is ~3x faster; used where
# Pool would finish too late or DVE has early slack)
G_TT_DVE = {(2, 1), (3, 1), (3, 2), (4, 1), (4, 2), (4, 3)}


def _is_kill(il, kind, im):
    """STT-drain (DVE) slots: fold the b0/q2 product into the drain."""
    if il == 3 and kind in ("dr", "di") and im < il:
        return True
    return il == 4 and (kind in ("dr0", "di0") or im <= 2)


def _pe_plan(il, kind, im):
    """-> (pe_products, fold or None); products are (col_key, rhs_ref),
    rhs_ref = (space, k) with space in fr/fi/gr/gi/d0r/d0i."""
    kill = _is_kill(il, kind, im)
    if kind == "dr0":
        pe = [("one", ("d0r", 0)), (("q1", il), ("fr", 1))]
        last = (("q2", il), ("fi", 1))
    elif kind == "di0":
        pe = [("one", ("d0i", 0)), (("r1", il), ("fi", 1))]
        last = (("q2", il), ("fr", 1))
    elif kind == "dr":
        if im < il and il in G_ROWS:
            pe = [("h1", ("gr", im)), ("h2n", ("gi", im))]
        elif im < il:
            pe = [("h1", ("fr", im - 1)), ("h2n", ("fi", im - 1)),
                  (("c1", il, im), ("fr", im + 1)),
                  (("c2n", il, im), ("fi", im + 1))]
        else:
            pe = [("h1", ("fr", im - 1)), ("h2n", ("fi", im - 1))]
        last = ((("b0p", im)), ("fi", im))
    elif kind == "di":
        if im < il and il in G_ROWS:
            pe = [("h1", ("gi", im)), ("h2p", ("gr", im))]
        elif im < il:
            pe = [("h1", ("fi", im - 1)), ("h2p", ("fr", im - 1)),
                  (("c1", il, im), ("fi", im + 1)),
                  (("c2p", il, im), ("fr", im + 1))]
        else:
            pe = [("h1", ("fi", im - 1)), ("h2p", ("fr", im - 1))]
        last = ((("b0n", im)), ("fr", im))
    else:
        raise ValueError(kind)
    if kill:
        return pe, last
    return pe + [last], None


# diag keys used on PE, in first-use order
DIAG_ORDER = []
for _s in SLOTS:
    for _ck, _ in _pe_plan(*_s)[0]:
        if _ck not in DIAG_ORDER:
            DIAG_ORDER.append(_ck)
NDIAG = len(DIAG_ORDER)
DIAG_POS = {k: i for i, k in enumerate(DIAG_ORDER)}
NDIAG_DVE = 14           # first N built on DVE, rest on ACT

# packed input layout (bf16): [identity (128 cols) | row blocks il=1..4]
# row block: fr slots (ns), fi slots (ns), d0r, d0i  -- each slot NV cols
IN_OFF = {}
_o = 128
for _il in range(1, L1):
    IN_OFF[_il] = _o
    _o += (2 * (_il + 1) + 2) * NV
CIN = _o

# packed output layout (bf16): row blocks il=1..4, pair-interleaved:
# [dr0, di0, dr1, di1, ...]
OUT_OFF = {}
_o = 0
for _il in range(1, L1):
    OUT_OFF[_il] = _o
    _o += 2 * (_il + 1) * NV
COUT = _o


# ---------------------------------------------------------------------------
# walrus in this container rejects >1 sync-wait per instruction; hoist
# extras onto same-engine NOPs.
def split_multiwaits(nc):
    for f in nc.m.functions:
        for blk in f.blocks:
            new = []
            changed = False
            for ins in blk.instructions:
                si = ins.sync_info
                if si is not None and len(si.on_wait) > 1:
                    waits = list(si.on_wait)
                    for w in waits[:-1]:
                        nop = mybir.InstNoOp(
                            name=nc.get_next_instruction_name(),
                            engine=ins.engine,
                            bass_nofuse=True,
                            sync_info=mybir.SyncInfo(on_wait=[w],
                                                     on_update=[]),
                        )
                        new.append(nop)
                    ins.sync_info = bass_rust.SyncInfo(
                        on_wait=[waits[-1]], on_update=list(si.on_update))
                    changed = True
                new.append(ins)
            if changed:
                blk.instructions = new


def _pair(ap, step_elems, nblocks=2):
    """Contiguous [P, L] AP -> [P, nblocks, L] with element step between
    blocks."""
    c = ap.copy()
    v = c.ap
    last = v.pop()
    v.append((step_elems, nblocks))
    v.append(tuple(last))
    c.ap = v
    return c


# ---------------------------------------------------------------------------
def build_bass(split=True):
    nc = bass.Bass()
    pin = nc.dram_tensor("pin", [XS, CIN], BF16, kind="ExternalInput").ap()
    psc = nc.dram_tensor("psc", [XS, NSCAL], F32, kind="ExternalInput").ap()
    pout = nc.dram_tensor("pout", [XS, COUT], BF16, kind="ExternalOutput").ap()

    with tile.TileContext(nc) as tc:
        with tc.tile_pool(name="m", bufs=1) as pool, \
             tc.tile_pool(name="p", bufs=1, space="PSUM") as ppool:
            scal = pool.tile([XS, NSCAL], F32, tag="scal")
            ident = pool.tile([XS, 128], BF16, tag="ident")
            diags = pool.tile([XS, NDIAG * 128], BF16, tag="diags")

            # issue order doubles as queue priority: scal, ident, rows 1..4
            nc.sync.dma_start(scal[:], psc[:])
            nc.sync.dma_start(ident[:], pin[:, 0:128])

            row_in = {}
            for il in range(1, L1):
                ns = il + 1
                t = pool.tile([XS, (2 * ns + 2) * NV], BF16,
                              name=f"in{il}", tag=f"in{il}")
                o = IN_OFF[il]
                if il == 1:
                    # priority part [fr1, fi1, d0r, d0i] lands first so the
                    # im=0 chains can start ~1.5us earlier
                    nc.sync.dma_start(t[:, 0:4 * NV], pin[:, o:o + 4 * NV])
                    nc.sync.dma_start(t[:, 4 * NV:6 * NV],
                                      pin[:, o + 4 * NV:o + 6 * NV])
                else:
                    nc.sync.dma_start(t[:],
                                      pin[:, o:o + (2 * ns + 2) * NV])
                row_in[il] = t

            # on-device diagonal weights: diag(col c) = ident * scal[:, c]
            for i, ckey in enumerate(DIAG_ORDER):
                dst = diags[:, i * 128:(i + 1) * 128]
                sc = scal[:, COL[ckey]:COL[ckey] + 1]
                if i < NDIAG_DVE:
                    nc.vector.tensor_scalar_mul(dst, ident[:], sc)
                else:
                    nc.scalar.mul(dst, ident[:], sc)

            # G tiles for rows 3-4 inner slots: G = F[im-1] + 2cB*F[im+1]
            # feed (DVE, imm scalar): P = 2cB * F[im+1]; add (Pool): G = P+F
            g_tile = {}
            p_tile = {}
            for il in G_ROWS:
                ni = il - 1
                g_tile[il] = pool.tile([XS, 2 * ni * NV], BF16,
                                       name=f"g{il}", tag=f"g{il}")
                p_tile[il] = pool.tile([XS, 2 * ni * NV], BF16,
                                       name=f"p{il}", tag=f"p{il}")
            for (il, im) in G_PAIRS:
                ni = il - 1
                ns = il + 1
                t = row_in[il]
                S = ns * NV
                Sg = ni * NV

                def fslot(k):
                    return t[:, k * NV:(k + 1) * NV]

                pslot = p_tile[il][:, (im - 1) * NV:im * NV]
                gslot = g_tile[il][:, (im - 1) * NV:im * NV]
                nc.vector.tensor_scalar_mul(
                    _pair(pslot, Sg), _pair(fslot(im + 1), S),
                    2.0 * _cB(il, im))
                eng = nc.vector if (il, im) in G_TT_DVE else nc.gpsimd
                eng.tensor_tensor(
                    _pair(gslot, Sg), _pair(pslot, Sg),
                    _pair(fslot(im - 1), S), ADD)

            # PE warm-up: ramp the pstate before row-1 data arrives.
            # scratch is never written: the values are irrelevant.
            scratch = pool.tile([XS, NV], BF16, tag="scratch")
            nc.gpsimd.memset(scratch[:], 0)
            wbank = ppool.tile([XS, NV], F32, name="wbank", tag="wbank")
            for _ in range(6):
                nc.tensor.matmul(wbank[:], scratch[:, 0:128], scratch[:],
                                 start=True, stop=True)

            def dg(ckey):
                i = DIAG_POS[ckey]
                return diags[:, i * 128:(i + 1) * 128]

            ROW1_SLOT = {("fr", 1): 0, ("fi", 1): 1, ("d0r", 0): 2,
                         ("d0i", 0): 3, ("fr", 0): 4, ("fi", 0): 5}

            def rhs_ap(il, ref):
                kind, k = ref
                if kind in ("gr", "gi"):
                    ni = il - 1
                    s = (k - 1) if kind == "gr" else (ni + k - 1)
                    return g_tile[il][:, s * NV:(s + 1) * NV]
                t = row_in[il]
                if il == 1:
                    s = ROW1_SLOT[(kind, k)]
                else:
                    ns = il + 1
                    base = {"fr": 0, "fi": ns,
                            "d0r": 2 * ns, "d0i": 2 * ns + 1}
                    s = base[kind] + k
                return t[:, s * NV:(s + 1) * NV]

            row_out = {}
            for il in range(1, L1):
                ns = il + 1
                row_out[il] = pool.tile([XS, 2 * ns * NV], BF16,
                                        name=f"out{il}", tag=f"out{il}")

            def out_ap(il, kind, im):
                # pair-interleaved: slot index = 2*im + (0 if dr else 1)
                s = 2 * im + (0 if kind.startswith("dr") else 1)
                return row_out[il][:, s * NV:(s + 1) * NV]

            banks = [ppool.tile([XS, NV], F32, name=f"bank{i}",
                                tag=f"bank{i}") for i in range(7)]
            banks.append(wbank)

            # waves aligned to input rows (row 4 split to fit 8 banks), so
            # diag-grouped matmul order never blocks on a later row's DMA
            waves = []
            for il in range(1, L1):
                rs = [s for s in SLOTS if s[0] == il]
                while rs:
                    waves.append(rs[:8])
                    rs = rs[8:]
            bank_no = 0
            for wave in waves:
                wbanks = []
                for _ in wave:
                    wbanks.append(banks[bank_no % 8])
                    bank_no += 1
                mms = []          # (diag_pos, slot_idx, ckey, il, ref)
                plans = []
                for j, (il, kind, im) in enumerate(wave):
                    pe, fold = _pe_plan(il, kind, im)
                    plans.append((il, kind, im, fold))
                    for ckey, ref in pe:
                        mms.append((DIAG_POS[ckey], j, ckey, il, ref))
                mms.sort(key=lambda m: m[0])
                remaining = [sum(1 for m in mms if m[1] == j)
                             for j in range(len(wave))]
                started = [False] * len(wave)
                for _, j, ckey, il, ref in mms:
                    remaining[j] -= 1
                    nc.tensor.matmul(wbanks[j][:], dg(ckey), rhs_ap(il, ref),
                                     start=not started[j],
                                     stop=remaining[j] == 0)
                    started[j] = True
                # drains + pair DMAs in slot order
                for j, (il, kind, im, fold) in enumerate(plans):
                    dst = out_ap(il, kind, im)
                    if fold is not None:
                        ckey, ref = fold
                        c = COL[ckey]
                        nc.vector.scalar_tensor_tensor(
                            dst, rhs_ap(il, ref), scal[:, c:c + 1],
                            wbanks[j][:], MULT, ADD)
                    else:
                        nc.scalar.copy(dst, wbanks[j][:])
                    if kind in ("di0", "di"):
                        o = OUT_OFF[il] + 2 * im * NV
                        nc.sync.dma_start(
                            pout[:, o:o + 2 * NV],
                            row_out[il][:, 2 * im * NV:(2 * im + 2) * NV])

    if split:
        split_multiwaits(nc)
    return nc


# ---------------------------------------------------------------------------
def pack_inputs(prev_f_re, prev_f_im, delta0_re, delta0_im, b):
    """-> per-core {'pin': [XS, CIN] bf16, 'psc': [XS, NSCAL] f32}."""
    in_maps = []
    for c in range(NCORES):
        X = slice(c * XS, (c + 1) * XS)
        b_sh = np.asarray(b[X], np.float32)
        p = np.zeros((XS, CIN), NPBF16)
        p[:, 0:128] = np.eye(XS, dtype=NPBF16)
        def bf(a):
            return np.asarray(a, np.float32).astype(NPBF16)

        # row 1 slot order matches ROW1_SLOT in build_bass
        o = IN_OFF[1]
        for i, a in enumerate([prev_f_re[1, 1, X], prev_f_im[1, 1, X],
                               delta0_re[1, 0, X], delta0_im[1, 0, X],
                               prev_f_re[1, 0, X], prev_f_im[1, 0, X]]):
            p[:, o + i * NV:o + (i + 1) * NV] = bf(a)
        for il in range(2, L1):
            o = IN_OFF[il]
            ns = il + 1
            p[:, o:o + ns * NV] = (
                np.asarray(prev_f_re[il, :ns, X, :], np.float32)
                .transpose(1, 0, 2).reshape(XS, ns * NV).astype(NPBF16))
            o += ns * NV
            p[:, o:o + ns * NV] = (
                np.asarray(prev_f_im[il, :ns, X, :], np.float32)
                .transpose(1, 0, 2).reshape(XS, ns * NV).astype(NPBF16))
            o += ns * NV
            p[:, o:o + NV] = np.asarray(
                delta0_re[il, 0, X, :], np.float32).astype(NPBF16)
            o += NV
            p[:, o:o + NV] = np.asarray(
                delta0_im[il, 0, X, :], np.float32).astype(NPBF16)
        in_maps.append({"pin": p, "psc": build_scal(b_sh)})
    return in_maps


def unpack_outputs(results, delta0_re, delta0_im):
    out = np.zeros((L1, L1, NX, NV), np.complex64)
    out[0, 0] = np.asarray(delta0_re[0, 0]) + 1j * np.asarray(delta0_im[0, 0])
    for c in range(NCORES):
        X = slice(c * XS, (c + 1) * XS)
        p = results[c]["pout"].astype(np.float32)
        for il in range(1, L1):
            o = OUT_OFF[il]
            ns = il + 1
            blk = p[:, o:o + 2 * ns * NV].reshape(XS, ns, 2, NV)
            dr = blk[:, :, 0, :].transpose(1, 0, 2)
            di = blk[:, :, 1, :].transpose(1, 0, 2)
            out[il, :ns, X, :] = dr + 1j * di
    return out


_NC_CACHE = None


def get_nc():
    global _NC_CACHE
    if _NC_CACHE is None:
        _NC_CACHE = build_bass()
    return _NC_CACHE


def kernel(prev_f_re, prev_f_im, delta0_re, delta0_im, b, v):
    in_maps = pack_inputs(prev_f_re, prev_f_im, delta0_re, delta0_im, b)
    res = run_bass_kernel_spmd(get_nc(), in_maps, list(range(NCORES)))
    return unpack_outputs(res.results, delta0_re, delta0_im)


# revision 14
# speedup vs baseline: 1.0640x; 1.0152x over previous
"""Trainium2 Bass kernel for nn_Bdfdv_51170240364850 (gnn_message_passing).

Computes, for mode pairs (il, im) with im <= il (L1 = 5 modes each way) and
grid (nx=1024, nv=512):

  D[il,im] = base + (-1j)*im*bx*F[il,im] + cB*bm*F[il,im+1]
             + [im==0] Re(cC*bp*F[il,1])
  base     = 0.5*bm*F[il,im-1]  (il>=1, 1<=im<=il)   else  D0[il,im]

with bx = b[:,0], bm = b[:,1]+1j b[:,2], bp = conj(bm),
cB = -(il-im)(il+im+1)/2, cC = -il(il+1).

Strategy: pure data-parallel over nx across 8 NeuronCores (nx=128 per core on
the 128 SBUF partitions), bf16 end-to-end (tolerance 2e-2; bf16 keeps
~7e-3). Per-x scalar products run on the Tensor engine as diagonal-weight
matmuls accumulating in PSUM:

    out[x, :] += c(x) * in[x, :]   ==   PSUM += diag(c).T @ in

Each of the 28 valid output slots (dr/di per (il, im), row 0 on host) is one
PSUM-bank chain of 2-5 matmuls (N=512). Work split that keeps every engine
busy:

 - PE: diagonal matmuls (74 + warm-up), diagonals built on-device from an
   identity tile x f32 coefficient columns (DVE/ACT tensor_scalar, ~200ns).
 - DVE: diag builds, G-feed tensor_scalar ops, and scalar_tensor_tensor
   drains for rows 3-4 that FOLD one product (b0/q2 term) into the
   PSUM->SBUF drain, cutting PE work.
 - ACT: remaining diag builds + plain copy drains (rows 1-2, row 3 im=0 and
   edge).
 - Pool (GpSimd): builds G = F[im-1] + 2cB*F[im+1] (constant integer
   coefficients) for rows 3-4 inner slots via tensor_tensor adds, so each
   inner slot needs only 2 matmuls (h1*G, -/+h2*G~) instead of 4.

Matmuls are emitted in waves of 7 PSUM banks, grouped by diagonal within a
wave to maximize stationary-weight reuse. Output slots are pair-interleaved
and DMA out per (dr,di) pair as soon as drained.
"""

import numpy as np
import ml_dtypes

import bass_rust
import concourse.bass as bass
import concourse.tile as tile
from concourse import mybir
from concourse.bass_utils import run_bass_kernel_spmd

L1 = 5
NX = 1024
NV = 512
NCORES = 8
XS = NX // NCORES  # 128 = SBUF partitions

F32 = mybir.dt.float32
BF16 = mybir.dt.bfloat16
NPBF16 = ml_dtypes.bfloat16

MULT = mybir.AluOpType.mult
ADD = mybir.AluOpType.add

# ---------------------------------------------------------------------------
# coefficient registry
PAIRS = [(2, 1), (3, 1), (3, 2), (4, 1), (4, 2), (4, 3)]  # inner (il, im)


def _cB(il, im):
    return -(il - im) * (il + im + 1) / 2.0


COL = {"one": 0, "h1": 1, "h2p": 2, "h2n": 3}
for _il in range(1, L1):
    COL[("q1", _il)] = 3 + _il           # 4..7
    COL[("r1", _il)] = 7 + _il           # 8..11
    COL[("q2", _il)] = 11 + _il          # 12..15
for _m in range(1, L1):
    COL[("b0p", _m)] = 15 + _m           # 16..19
    COL[("b0n", _m)] = 19 + _m           # 20..23
for _k, _p in enumerate(PAIRS):
    COL[("c1",) + _p] = 24 + 3 * _k
    COL[("c2p",) + _p] = 25 + 3 * _k
    COL[("c2n",) + _p] = 26 + 3 * _k
NSCAL = 48  # padded f32 table width


def coeff_vec(key, b0, b1, b2):
    if key == "one":
        return np.ones_like(b0)
    if key == "h1":
        return 0.5 * b1
    if key == "h2p":
        return 0.5 * b2
    if key == "h2n":
        return -0.5 * b2
    tag = key[0]
    if tag == "q1":
        return 3.0 * _cB(key[1], 0) * b1
    if tag == "r1":
        return _cB(key[1], 0) * b1
    if tag == "q2":
        return _cB(key[1], 0) * b2
    if tag == "b0p":
        return key[1] * b0
    if tag == "b0n":
        return -key[1] * b0
    cB = _cB(key[1], key[2])
    if tag == "c1":
        return cB * b1
    if tag == "c2p":
        return cB * b2
    if tag == "c2n":
        return -cB * b2
    raise ValueError(key)


def build_scal(b_sh):
    """b_sh: [XS, 3] f32 -> [XS, NSCAL] f32 (STT-drain scalars by COL)."""
    b0, b1, b2 = b_sh[:, 0], b_sh[:, 1], b_sh[:, 2]
    s = np.zeros((XS, NSCAL), np.float32)
    for key, c in COL.items():
        s[:, c] = coeff_vec(key, b0, b1, b2)
    return s


# ---------------------------------------------------------------------------
# slot schedule: which products run on PE, which fold into the DVE drain,
# and which inner slots read the Pool-built G tiles.
SLOTS = []
for _il in range(1, L1):
    SLOTS.append((_il, "dr0", 0))
    SLOTS.append((_il, "di0", 0))
    for _im in range(1, _il + 1):
        SLOTS.append((_il, "dr", _im))
        SLOTS.append((_il, "di", _im))

G_ROWS = (2, 3, 4)       # rows whose inner slots use G tiles
G_PAIRS = [(il, im) for il in G_ROWS for im in range(1, il)]
# G adds on Pool except these (DVE tensor_tensor is ~3x faster; used where
# Pool would finish too late or DVE has early slack)
G_TT_DVE = {(2, 1), (3, 1), (3, 2), (4, 1), (4, 2), (4, 3)}


def _is_kill(il, kind, im):
    """STT-drain (DVE) slots: fold the b0/q2 product into the drain."""
    if il == 3 and kind in ("dr", "di") and im < il:
        return True
    return il == 4 and (kind in ("dr0", "di0") or im <= 2)


def _pe_plan(il, kind, im):
    """-> (pe_products, fold or None); products are (col_key, rhs_ref),
    rhs_ref = (space, k) with space in fr/fi/gr/gi/d0r/d0i."""
    kill = _is_kill(il, kind, im)
    if kind == "dr0":
        pe = [("one", ("d0r", 0)), (("q1", il), ("fr", 1))]
        last = (("q2", il), ("fi", 1))
    elif kind == "di0":
        pe = [("one", ("d0i", 0)), (("r1", il), ("fi", 1))]
        last = (("q2", il), ("fr", 1))
    elif kind == "dr":
        if im < il and il in G_ROWS:
            pe = [("h1", ("gr", im)), ("h2n", ("gi", im))]
        elif im < il:
            pe = [("h1", ("fr", im - 1)), ("h2n", ("fi", im - 1)),
                  (("c1", il, im), ("fr", im + 1)),
                  (("c2n", il, im), ("fi", im + 1))]
        else:
            pe = [("h1", ("fr", im - 1)), ("h2n", ("fi", im - 1))]
        last = ((("b0p", im)), ("fi", im))
    elif kind == "di":
        if im < il and il in G_ROWS:
            pe = [("h1", ("gi", im)), ("h2p", ("gr", im))]
        elif im < il:
            pe = [("h1", ("fi", im - 1)), ("h2p", ("fr", im - 1)),
                  (("c1", il, im), ("fi", im + 1)),
                  (("c2p", il, im), ("fr", im + 1))]
        else:
            pe = [("h1", ("fi", im - 1)), ("h2p", ("fr", im - 1))]
        last = ((("b0n", im)), ("fr", im))
    else:
        raise ValueError(kind)
    if kill:
        return pe, last
    return pe + [last], None


# diag keys used on PE, in first-use order
DIAG_ORDER = []
for _s in SLOTS:
    for _ck, _ in _pe_plan(*_s)[0]:
        if _ck not in DIAG_ORDER:
            DIAG_ORDER.append(_ck)
NDIAG = len(DIAG_ORDER)
DIAG_POS = {k: i for i, k in enumerate(DIAG_ORDER)}
NDIAG_DVE = 14           # first N built on DVE, rest on ACT

# packed input layout (bf16): [identity (128 cols) | row blocks il=1..4]
# row block: fr slots (ns), fi slots (ns), d0r, d0i  -- each slot NV cols
IN_OFF = {}
_o = 128
for _il in range(1, L1):
    IN_OFF[_il] = _o
    _o += (2 * (_il + 1) + 2) * NV
CIN = _o

# packed output layout (bf16): row blocks il=1..4, pair-interleaved:
# [dr0, di0, dr1, di1, ...]
OUT_OFF = {}
_o = 0
for _il in range(1, L1):
    OUT_OFF[_il] = _o
    _o += 2 * (_il + 1) * NV
COUT = _o


# ---------------------------------------------------------------------------
# walrus in this container rejects >1 sync-wait per instruction; hoist
# extras onto same-engine NOPs.
def split_multiwaits(nc):
    for f in nc.m.functions:
        for blk in f.blocks:
            new = []
            changed = False
            for ins in blk.instructions:
                si = ins.sync_info
                if si is not None and len(si.on_wait) > 1:
                    waits = list(si.on_wait)
                    for w in waits[:-1]:
                        nop = mybir.InstNoOp(
                            name=nc.get_next_instruction_name(),
                            engine=ins.engine,
                            bass_nofuse=True,
                            sync_info=mybir.SyncInfo(on_wait=[w],
                                                     on_update=[]),
                        )
                        new.append(nop)
                    ins.sync_info = bass_rust.SyncInfo(
                        on_wait=[waits[-1]], on_update=list(si.on_update))
                    changed = True
                new.append(ins)
            if changed:
                blk.instructions = new


def _pair(ap, step_elems, nblocks=2):
    """Contiguous [P, L] AP -> [P, nblocks, L] with element step between
    blocks."""
    c = ap.copy()
    v = c.ap
    last = v.pop()
    v.append((step_elems, nblocks))
    v.append(tuple(last))
    c.ap = v
    return c


# ---------------------------------------------------------------------------
def build_bass(split=True):
    nc = bass.Bass()
    pin = nc.dram_tensor("pin", [XS, CIN], BF16, kind="ExternalInput").ap()
    psc = nc.dram_tensor("psc", [XS, NSCAL], F32, kind="ExternalInput").ap()
    pout = nc.dram_tensor("pout", [XS, COUT], BF16, kind="ExternalOutput").ap()

    with tile.TileContext(nc) as tc:
        with tc.tile_pool(name="m", bufs=1) as pool, \
             tc.tile_pool(name="p", bufs=1, space="PSUM") as ppool:
            scal = pool.tile([XS, NSCAL], F32, tag="scal")
            ident = pool.tile([XS, 128], BF16, tag="ident")
            diags = pool.tile([XS, NDIAG * 128], BF16, tag="diags")

            # issue order doubles as queue priority: scal, ident, rows 1..4
            nc.sync.dma_start(scal[:], psc[:])
            nc.sync.dma_start(ident[:], pin[:, 0:128])

            row_in = {}
            for il in range(1, L1):
                ns = il + 1
                t = pool.tile([XS, (2 * ns + 2) * NV], BF16,
                              name=f"in{il}", tag=f"in{il}")
                o = IN_OFF[il]
                if il == 1:
                    # priority part [fr1, fi1, d0r, d0i] lands first so the
                    # im=0 chains can start ~1.5us earlier
                    nc.sync.dma_start(t[:, 0:4 * NV], pin[:, o:o + 4 * NV])
                    nc.sync.dma_start(t[:, 4 * NV:6 * NV],
                                      pin[:, o + 4 * NV:o + 6 * NV])
                else:
                    # f-slots first: G feeds and h-products unblock before
                    # the d0 tail arrives
                    nc.sync.dma_start(t[:, 0:2 * ns * NV],
                                      pin[:, o:o + 2 * ns * NV])
                    nc.sync.dma_start(
                        t[:, 2 * ns * NV:(2 * ns + 2) * NV],
                        pin[:, o + 2 * ns * NV:o + (2 * ns + 2) * NV])
                row_in[il] = t

            # on-device diagonal weights: diag(col c) = ident * scal[:, c]
            # first 10 alternate across DVE/ACT so wave-1's set is ready
            # sooner; the rest go to whichever is cheaper (DVE) vs idle (ACT)
            for i, ckey in enumerate(DIAG_ORDER):
                dst = diags[:, i * 128:(i + 1) * 128]
                sc = scal[:, COL[ckey]:COL[ckey] + 1]
                if i < 10:
                    on_dve = (i % 2 == 0)
                else:
                    on_dve = i < NDIAG_DVE
                if on_dve:
                    nc.vector.tensor_scalar_mul(dst, ident[:], sc)
                else:
                    nc.scalar.mul(dst, ident[:], sc)

            # G tiles for rows 3-4 inner slots: G = F[im-1] + 2cB*F[im+1]
            # feed (DVE, imm scalar): P = 2cB * F[im+1]; add (Pool): G = P+F
            g_tile = {}
            p_tile = {}
            for il in G_ROWS:
                ni = il - 1
                g_tile[il] = pool.tile([XS, 2 * ni * NV], BF16,
                                       name=f"g{il}", tag=f"g{il}")
                p_tile[il] = pool.tile([XS, 2 * ni * NV], BF16,
                                       name=f"p{il}", tag=f"p{il}")
            for (il, im) in G_PAIRS:
                ni = il - 1
                ns = il + 1
                t = row_in[il]
                S = ns * NV
                Sg = ni * NV

                def fslot(k):
                    return t[:, k * NV:(k + 1) * NV]

                pslot = p_tile[il][:, (im - 1) * NV:im * NV]
                gslot = g_tile[il][:, (im - 1) * NV:im * NV]
                nc.vector.tensor_scalar_mul(
                    _pair(pslot, Sg), _pair(fslot(im + 1), S),
                    2.0 * _cB(il, im))
                eng = nc.vector if (il, im) in G_TT_DVE else nc.gpsimd
                eng.tensor_tensor(
                    _pair(gslot, Sg), _pair(pslot, Sg),
                    _pair(fslot(im - 1), S), ADD)

            # PE warm-up: ramp the pstate before row-1 data arrives.
            # scratch is never written: the values are irrelevant.
            scratch = pool.tile([XS, NV], BF16, tag="scratch")
            nc.gpsimd.memset(scratch[:], 0)
            wbank = ppool.tile([XS, NV], F32, name="wbank", tag="wbank")
            for _ in range(6):
                nc.tensor.matmul(wbank[:], scratch[:, 0:128], scratch[:],
                                 start=True, stop=True)

            def dg(ckey):
                i = DIAG_POS[ckey]
                return diags[:, i * 128:(i + 1) * 128]

            ROW1_SLOT = {("fr", 1): 0, ("fi", 1): 1, ("d0r", 0): 2,
                         ("d0i", 0): 3, ("fr", 0): 4, ("fi", 0): 5}

            def rhs_ap(il, ref):
                kind, k = ref
                if kind in ("gr", "gi"):
                    ni = il - 1
                    s = (k - 1) if kind == "gr" else (ni + k - 1)
                    return g_tile[il][:, s * NV:(s + 1) * NV]
                t = row_in[il]
                if il == 1:
                    s = ROW1_SLOT[(kind, k)]
                else:
                    ns = il + 1
                    base = {"fr": 0, "fi": ns,
                            "d0r": 2 * ns, "d0i": 2 * ns + 1}
                    s = base[kind] + k
                return t[:, s * NV:(s + 1) * NV]

            row_out = {}
            for il in range(1, L1):
                ns = il + 1
                row_out[il] = pool.tile([XS, 2 * ns * NV], BF16,
                                        name=f"out{il}", tag=f"out{il}")

            def out_ap(il, kind, im):
                # pair-interleaved: slot index = 2*im + (0 if dr else 1)
                s = 2 * im + (0 if kind.startswith("dr") else 1)
                return row_out[il][:, s * NV:(s + 1) * NV]

            banks = [ppool.tile([XS, NV], F32, name=f"bank{i}",
                                tag=f"bank{i}") for i in range(7)]
            banks.append(wbank)

            # waves aligned to input rows (row 4 split to fit 8 banks), so
            # diag-grouped matmul order never blocks on a later row's DMA
            waves = []
            for il in range(1, L1):
                rs = [s for s in SLOTS if s[0] == il]
                while rs:
                    waves.append(rs[:8])
                    rs = rs[8:]
            bank_no = 0
            for wave in waves:
                wbanks = []
                for _ in wave:
                    wbanks.append(banks[bank_no % 8])
                    bank_no += 1
                mms = []          # (diag_pos, slot_idx, ckey, il, ref)
                plans = []
                for j, (il, kind, im) in enumerate(wave):
                    pe, fold = _pe_plan(il, kind, im)
                    plans.append((il, kind, im, fold))
                    for ckey, ref in pe:
                        mms.append((DIAG_POS[ckey], j, ckey, il, ref))
                mms.sort(key=lambda m: m[0])
                remaining = [sum(1 for m in mms if m[1] == j)
                             for j in range(len(wave))]
                started = [False] * len(wave)
                for _, j, ckey, il, ref in mms:
                    remaining[j] -= 1
                    nc.tensor.matmul(wbanks[j][:], dg(ckey), rhs_ap(il, ref),
                                     start=not started[j],
                                     stop=remaining[j] == 0)
                    started[j] = True
                # drains + pair DMAs in slot order
                for j, (il, kind, im, fold) in enumerate(plans):
                    dst = out_ap(il, kind, im)
                    if fold is not None:
                        ckey, ref = fold
                        c = COL[ckey]
                        nc.vector.scalar_tensor_tensor(
                            dst, rhs_ap(il, ref), scal[:, c:c + 1],
                            wbanks[j][:], MULT, ADD)
                    else:
                        nc.scalar.copy(dst, wbanks[j][:])
                    if kind in ("di0", "di"):
                        o = OUT_OFF[il] + 2 * im * NV
                        nc.sync.dma_start(
                            pout[:, o:o + 2 * NV],
                            row_out[il][:, 2 * im * NV:(2 * im + 2) * NV])

    if split:
        split_multiwaits(nc)
    return nc


# ---------------------------------------------------------------------------
def pack_inputs(prev_f_re, prev_f_im, delta0_re, delta0_im, b):
    """-> per-core {'pin': [XS, CIN] bf16, 'psc': [XS, NSCAL] f32}."""
    in_maps = []
    for c in range(NCORES):
        X = slice(c * XS, (c + 1) * XS)
        b_sh = np.asarray(b[X], np.float32)
        p = np.zeros((XS, CIN), NPBF16)
        p[:, 0:128] = np.eye(XS, dtype=NPBF16)
        def bf(a):
            return np.asarray(a, np.float32).astype(NPBF16)

        # row 1 slot order matches ROW1_SLOT in build_bass
        o = IN_OFF[1]
        for i, a in enumerate([prev_f_re[1, 1, X], prev_f_im[1, 1, X],
                               delta0_re[1, 0, X], delta0_im[1, 0, X],
                               prev_f_re[1, 0, X], prev_f_im[1, 0, X]]):
            p[:, o + i * NV:o + (i + 1) * NV] = bf(a)
        for il in range(2, L1):
            o = IN_OFF[il]
            ns = il + 1
            p[:, o:o + ns * NV] = (
                np.asarray(prev_f_re[il, :ns, X, :], np.float32)
                .transpose(1, 0, 2).reshape(XS, ns * NV).astype(NPBF16))
            o += ns * NV
            p[:, o:o + ns * NV] = (
                np.asarray(prev_f_im[il, :ns, X, :], np.float32)
                .transpose(1, 0, 2).reshape(XS, ns * NV).astype(NPBF16))
            o += ns * NV
            p[:, o:o + NV] = np.asarray(
                delta0_re[il, 0, X, :], np.float32).astype(NPBF16)
            o += NV
            p[:, o:o + NV] = np.asarray(
                delta0_im[il, 0, X, :], np.float32).astype(NPBF16)
        in_maps.append({"pin": p, "psc": build_scal(b_sh)})
    return in_maps


def unpack_outputs(results, delta0_re, delta0_im):
    out = np.zeros((L1, L1, NX, NV), np.complex64)
    out[0, 0] = np.asarray(delta0_re[0, 0]) + 1j * np.asarray(delta0_im[0, 0])
    for c in range(NCORES):
        X = slice(c * XS, (c + 1) * XS)
        p = results[c]["pout"].astype(np.float32)
        for il in range(1, L1):
            o = OUT_OFF[il]
            ns = il + 1
            blk = p[:, o:o + 2 * ns * NV].reshape(XS, ns, 2, NV)
            dr = blk[:, :, 0, :].transpose(1, 0, 2)
            di = blk[:, :, 1, :].transpose(1, 0, 2)
            out[il, :ns, X, :] = dr + 1j * di
    return out


_NC_CACHE = None


def get_nc():
    global _NC_CACHE
    if _NC_CACHE is None:
        _NC_CACHE = build_bass()
    return _NC_CACHE


def kernel(prev_f_re, prev_f_im, delta0_re, delta0_im, b, v):
    in_maps = pack_inputs(prev_f_re, prev_f_im, delta0_re, delta0_im, b)
    res = run_bass_kernel_spmd(get_nc(), in_maps, list(range(NCORES)))
    return unpack_outputs(res.results, delta0_re, delta0_im)
